# revision 10
# baseline (speedup 1.0000x reference)
"""Trainium2 Bass kernel for nn_BasicTJDLayer (tensor-train joint distribution layer).

Reference computation (all f32):
    g_t = core[:, y_t, :]                 (B,N) token gathers of (R,R) slices
    v   = alpha; v = v @ g_t  (N steps)   -> prob_tilde[b] = v @ beta
    M   = core.sum(axis=1); u = alpha @ M^N -> norm = u @ beta
    loss = mean(-log(prob_tilde+eps) + log(norm+eps)); prob = prob_tilde/norm

Distribution over 8 NeuronCores (per the sharding hint: data-parallel over
batch; label_ids and the gathered core chain sharded over B; the core table
vocab-sharded for the normalization sum):
  - Phase A (M = sum over vocab): vocab-sharded, 4000 entries/core. Streamed
    as (128, 4000) tiles; PE matmuls against a stacked identity reduce the
    4 vocab sub-blocks and accumulate in PSUM.
  - Phase B (token chains): batch-sharded, 2 batch rows/core. Each batch
    row's 1024-token chain is split into 64 segments of 16 tokens; segment
    products are computed on-device with 4 segments packed per 128-wide
    block-diagonal matmul (Q <- G^T Q). The host supplies the gathered
    chain (core[:, ys, :]); DMAs write only the diagonal 32x32 blocks of
    the per-step stationary tiles, whose off-diagonal zeros are memset
    once and never rewritten.
  - Host: assembles M from per-core partials, runs the exact sequential
    norm chain (matches jax.lax.scan order), and combines the 64 segment
    products per batch row (tiny O(B*S*R^2) glue).
"""

import numpy as np

R = 32
V = 32000
B = 16
N = 1024
EPS = np.float32(1e-10)

NCORES = 8
VS = V // NCORES            # 4000 vocab entries per core (phase A)
BS = B // NCORES            # 2 batch rows per core (phase B)
SEG = 16                    # tokens per segment
SEGS = N // SEG             # 64 segments per batch row
CHUNKS = 4                  # chain chunks per core = (batch row, half)
QG_PER_CHUNK = 8            # quad-groups (4 segments each) per chunk
TOK_PER_CJ = QG_PER_CHUNK * SEG           # 128 tokens per (chunk, j)
REGION_F = TOK_PER_CJ * R                 # 4096 f32 region per j per partition
SLOT_F = 4 * REGION_F                     # 16384 f32 per partition per slot

# Phase A tiling
PA_TILES = 8                # tiles per core
PA_V = VS // PA_TILES       # 500 vocab entries per tile
PA_VB = PA_V // 4           # 125 per partition-block
PA_F = PA_VB * R            # 4000 f32 free per partition
PA_CHUNKS = (PA_F + 127) // 128   # 32 matmul chunks per tile (31 full + 1 of 32)


def build_nc():
    from concourse import bass, bacc, mybir, tile

    f32 = mybir.dt.float32

    nc = bacc.Bacc(None, target_bir_lowering=False, debug=False)

    pa = nc.dram_tensor("pa", [R, VS, R], f32, kind="ExternalInput")
    chain = nc.dram_tensor("chain", [CHUNKS, 4, R, TOK_PER_CJ * R], f32,
                           kind="ExternalInput")
    m_out = nc.dram_tensor("m_out", [R, R], f32, kind="ExternalOutput")
    q_out = nc.dram_tensor("q_out", [CHUNKS * 2, 128, 128], f32,
                           kind="ExternalOutput")

    # istack[32*d + r, m] = (r == m): phase-A reducer (moving) and the
    # stacked identity Q_0 for the first chain step of every segment.
    istack_np = np.tile(np.eye(R, dtype=np.float32), (4, 1))
    istack_dram = nc.inline_tensor(istack_np, name="istack")

    with tile.TileContext(nc) as tc:
        with (
            tc.tile_pool(name="const", bufs=1) as constp,
            tc.tile_pool(name="pa_pool", bufs=2) as pap,
            tc.tile_pool(name="fold", bufs=1) as foldp,
            tc.tile_pool(name="stat", bufs=1) as statp,
            tc.tile_pool(name="qx", bufs=4) as qxp,
            tc.tile_pool(name="psA", bufs=1, space="PSUM") as psa,
            tc.tile_pool(name="psB", bufs=4, space="PSUM") as psb,
        ):
            istack = constp.tile([128, R], f32)
            nc.sync.dma_start(out=istack[:, :], in_=istack_dram[:, :])

            # ---------------- Phase A: M partial = sum_v pa[:, v, :] -------
            psumA = psa.tile([128, R], f32, space="PSUM")
            first_mm = True
            for t in range(PA_TILES):
                ta = pap.tile([128, PA_F], f32, tag="pa_tile")
                for d in range(4):
                    v0 = t * PA_V + d * PA_VB
                    nc.sync.dma_start(
                        out=ta[32 * d:32 * (d + 1), :],
                        in_=pa[:, v0:v0 + PA_VB, :],
                    )
                for m in range(PA_CHUNKS):
                    f0 = m * 128
                    f1 = min(f0 + 128, PA_F)
                    nc.tensor.matmul(
                        out=psumA[0:(f1 - f0), :],
                        lhsT=ta[:, f0:f1],
                        rhs=istack[:, :],
                        start=first_mm,
                        stop=(t == PA_TILES - 1 and m == PA_CHUNKS - 1),
                        skip_group_check=True,
                    )
                    first_mm = False

            # Fold the 4 partition blocks of psumA with one matmul:
            # out[s, r] = sum_p istack[p, s] * psumA_sbuf[p, r].
            aS = foldp.tile([128, R], f32)
            nc.vector.tensor_copy(out=aS[:, :], in_=psumA[:, :])
            psumM = psa.tile([R, R], f32, space="PSUM", tag="psumM")
            nc.tensor.matmul(out=psumM[:, :], lhsT=istack[:, :], rhs=aS[:, :],
                             start=True, stop=True)
            m_tile = foldp.tile([R, R], f32)
            nc.vector.tensor_copy(out=m_tile[:, :], in_=psumM[:, :])
            nc.sync.dma_start(out=m_out[:, :], in_=m_tile[:, :])

            # ---------------- Phase B: segment chains -----------------------
            # Two stationary ring slots, free layout (j, t, s): region j
            # occupies free [j*REGION_F, (j+1)*REGION_F) and only partitions
            # [32j, 32j+32) of it are ever written -> block-diagonal lhsT via
            # a 2-free-dim AP. Off-diagonal stays zero forever.
            slots = []
            for s in range(2):
                st = statp.tile([128, SLOT_F], f32, tag=f"slot{s}")
                third = SLOT_F // 4
                nc.vector.memset(st[:, 0:third], 0.0)
                nc.gpsimd.memset(st[:, third:2 * third], 0.0)
                nc.vector.memset(st[:, 2 * third:3 * third], 0.0)
                nc.gpsimd.memset(st[:, 3 * third:], 0.0)
                slots.append(st)

            for c in range(CHUNKS):
                st = slots[c % 2]
                # Slot free layout (t, j, s): step-tile t is the contiguous
                # 128-f32 slice [t*128, (t+1)*128) = [Z|..|G_j|..|Z] block-
                # diagonal row chunk (zeros persist from the one-time memset).
                stv = st[:, :].rearrange("p (t j s) -> p t j s",
                                         t=TOK_PER_CJ, j=4, s=R)
                # Load chunk c: 4 DMAs, each 32 partitions x 128 runs of 128B.
                for j in range(4):
                    nc.sync.dma_start(
                        out=stv[32 * j:32 * (j + 1), :, j, :],
                        in_=chain[c, j, :, :],
                    )

                # Chains: 2 q-quads (4 quad-groups each) per chunk.
                for g4 in range(2):
                    qprev = None
                    for i in range(SEG):
                        psq = psb.tile([128, 128], f32, space="PSUM", tag="psq")
                        for k in range(4):
                            qg = g4 * 4 + k
                            tok = qg * SEG + i
                            # Step tile: contiguous 128-wide block-diag lhsT.
                            lhsT = st[:, tok * 128:(tok + 1) * 128]
                            rhs = (istack[:, :] if i == 0
                                   else qprev[:, 32 * k:32 * (k + 1)])
                            nc.tensor.matmul(
                                out=psq[:, 32 * k:32 * (k + 1)],
                                lhsT=lhsT,
                                rhs=rhs,
                                start=True,
                                stop=True,
                            )
                        qnew = qxp.tile([128, 128], f32, tag="qq")
                        if i % 2 == 0:
                            nc.vector.tensor_copy(out=qnew[:, :], in_=psq[:, :])
                        else:
                            nc.scalar.copy(out=qnew[:, :], in_=psq[:, :])
                        qprev = qnew
                    nc.sync.dma_start(out=q_out[c * 2 + g4, :, :], in_=qprev[:, :])

    nc.compile()
    return nc


def _host_inputs(alpha, beta, core, label_ids):
    """Build per-core input maps. core: (R, V, R) f32; label_ids: (B, N) int."""
    core = np.ascontiguousarray(np.asarray(core, dtype=np.float32))
    lab = np.asarray(label_ids)

    in_maps = []
    for cidx in range(NCORES):
        pa = np.ascontiguousarray(core[:, cidx * VS:(cidx + 1) * VS, :])
        ch = np.empty((CHUNKS, 4, R, TOK_PER_CJ, R), dtype=np.float32)
        for c in range(CHUNKS):
            bb = c // 2
            half = c % 2
            b_global = BS * cidx + bb
            for j in range(4):
                segs = half * 32 + np.arange(QG_PER_CHUNK) * 4 + j
                pos = (segs[:, None] * SEG + np.arange(SEG)[None, :]).ravel()
                ys = lab[b_global, pos]
                ch[c, j] = core[:, ys, :]
        in_maps.append({
            "pa": pa,
            "chain": ch.reshape(CHUNKS, 4, R, TOK_PER_CJ * R),
        })
    return in_maps


def _host_finish(alpha, beta, m_parts, q_parts):
    """Combine per-core results into (loss, prob) with reference f32 semantics."""
    alpha = np.asarray(alpha, dtype=np.float32)
    beta = np.asarray(beta, dtype=np.float32)

    # M partial tiles are (s, r); sum cores then transpose to (r, s).
    M = np.zeros((R, R), dtype=np.float32)
    for mp in m_parts:
        M = M + np.asarray(mp).reshape(R, R).T.astype(np.float32)

    # Exact sequential norm chain (matches jax.lax.scan order).
    u = alpha.copy()
    for _ in range(N):
        u = (u @ M).astype(np.float32)
    norm = np.float32(u @ beta)

    # Segment products: q_parts[c] shape (8, 128, 128).
    prob_tilde = np.empty((B,), dtype=np.float32)
    with np.errstate(over="ignore", invalid="ignore"):
        for cidx in range(NCORES):
            qo = np.asarray(q_parts[cidx]).reshape(CHUNKS * 2, 128, 128)
            Q = {}
            for c in range(CHUNKS):
                bb = c // 2
                half = c % 2
                for g4 in range(2):
                    tileq = qo[c * 2 + g4]
                    for k in range(4):
                        qg = g4 * 4 + k
                        for j in range(4):
                            seg = half * 32 + qg * 4 + j
                            Q[(bb, seg)] = tileq[32 * j:32 * (j + 1),
                                                 32 * k:32 * (k + 1)]
            for bb in range(BS):
                v = alpha.copy()
                for seg in range(SEGS):
                    # Q_seg = P_seg^T ; v_row @ P_seg == Q_seg @ v_col
                    v = (Q[(bb, seg)] @ v).astype(np.float32)
                prob_tilde[BS * cidx + bb] = np.float32(v @ beta)

    with np.errstate(divide="ignore", invalid="ignore", over="ignore"):
        loss = np.float32(np.mean(-np.log(prob_tilde + EPS) + np.log(norm + EPS)))
        prob = (prob_tilde / norm).astype(np.float32)
    return loss, prob


_NC_CACHE = {}


def kernel(alpha, beta, core, label_ids):
    from concourse.bass_utils import run_bass_kernel_spmd

    if "nc" not in _NC_CACHE:
        _NC_CACHE["nc"] = build_nc()
    nc = _NC_CACHE["nc"]

    in_maps = _host_inputs(alpha, beta, core, label_ids)
    res = run_bass_kernel_spmd(nc, in_maps, core_ids=list(range(NCORES)))
    m_parts = [res.results[c]["m_out"] for c in range(NCORES)]
    q_parts = [res.results[c]["q_out"] for c in range(NCORES)]
    return _host_finish(alpha, beta, m_parts, q_parts)


# revision 18
# speedup vs baseline: 1.0232x; 1.0232x over previous
"""Trainium2 Bass kernel for nn_BasicTJDLayer (tensor-train joint distribution layer).

Reference computation (all f32):
    g_t = core[:, y_t, :]                 (B,N) token gathers of (R,R) slices
    v   = alpha; v = v @ g_t  (N steps)   -> prob_tilde[b] = v @ beta
    M   = core.sum(axis=1); u = alpha @ M^N -> norm = u @ beta
    loss = mean(-log(prob_tilde+eps) + log(norm+eps)); prob = prob_tilde/norm

Distribution over 8 NeuronCores (per the sharding hint: data-parallel over
batch; label_ids and the gathered core chain sharded over B; the core table
vocab-sharded for the normalization sum):
  - Phase A (M = sum over vocab): vocab-sharded, 4000 entries/core. Streamed
    as (128, 4000) tiles; PE matmuls against a stacked identity reduce the
    4 vocab sub-blocks, accumulating round-robin over 4 PSUM banks.
  - Phase B (token chains): batch-sharded, 2 batch rows/core. Each batch
    row's 1024-token chain is split into 64 segments of 16 tokens; segment
    products are computed on-device with 4 segments packed per 128-wide
    block-diagonal matmul (Q <- G^T Q). The host supplies the gathered
    chain (core[:, ys, :]); DMAs write only the diagonal 32x32 blocks of
    the per-step stationary tiles, whose off-diagonal zeros are memset
    once and never rewritten. Four step-chains run in lockstep (two
    chunks x two quad-quads), with PSUM->SBUF copies split DVE/ACT.
  - Host: assembles M from per-core partials, runs the exact sequential
    norm chain (matches jax.lax.scan order), and combines the 64 segment
    products per batch row (tiny O(B*S*R^2) glue).
"""

import numpy as np

R = 32
V = 32000
B = 16
N = 1024
EPS = np.float32(1e-10)

NCORES = 8
VS = V // NCORES            # 4000 vocab entries per core (phase A)
BS = B // NCORES            # 2 batch rows per core (phase B)
SEG = 16                    # tokens per segment
SEGS = N // SEG             # 64 segments per batch row
CHUNKS = 4                  # chain chunks per core = (batch row, half)
QG_PER_CHUNK = 8            # quad-groups (4 segments each) per chunk
TOK_PER_CJ = QG_PER_CHUNK * SEG           # 128 tokens per (chunk, j)
SLOT_F = TOK_PER_CJ * 128                 # 16384 f32 per partition per slot

# Phase A tiling
PA_TILES = 8                # tiles per core
PA_V = VS // PA_TILES       # 500 vocab entries per tile
PA_VB = PA_V // 4           # 125 per partition-block
PA_F = PA_VB * R            # 4000 f32 free per partition
PA_CHUNKS = (PA_F + 127) // 128   # 32 matmul chunks per tile (31 full + 1 of 32)
PA_ACC = 4                  # round-robin PSUM accumulators (new_pa)


def build_nc(bench_reps=None, new_pa=False, new_chain=True):
    """Build the SPMD program. bench_reps=None -> real kernel (external I/O);
    bench_reps=K -> timing variant: body wrapped in For_i(K) over Internal
    DRAM scratch, with a trivial external in/out pair."""
    from concourse import bass, bacc, mybir, tile
    from contextlib import nullcontext

    f32 = mybir.dt.float32
    bench = bench_reps is not None

    nc = bacc.Bacc(None, target_bir_lowering=False, debug=False)

    if bench:
        tick = nc.dram_tensor("tick", [1, 1], f32, kind="ExternalInput")
        pa = nc.dram_tensor("pa", [R, VS, R], f32)
        chain = nc.dram_tensor("chain", [CHUNKS, 4, R, TOK_PER_CJ * R], f32)
        m_out = nc.dram_tensor("m_out", [R, R], f32)
        q_out = nc.dram_tensor("q_out", [CHUNKS * 2, 128, 128], f32)
        done = nc.dram_tensor("done", [1, 1], f32, kind="ExternalOutput")
    else:
        pa = nc.dram_tensor("pa", [R, VS, R], f32, kind="ExternalInput")
        chain = nc.dram_tensor("chain", [CHUNKS, 4, R, TOK_PER_CJ * R], f32,
                               kind="ExternalInput")
        m_out = nc.dram_tensor("m_out", [R, R], f32, kind="ExternalOutput")
        q_out = nc.dram_tensor("q_out", [CHUNKS * 2, 128, 128], f32,
                               kind="ExternalOutput")

    # istack[32*d + r, m] = (r == m): phase-A reducer (moving) and the
    # stacked identity Q_0 for the first chain step of every segment.
    istack_np = np.tile(np.eye(R, dtype=np.float32), (4, 1))
    istack_dram = nc.inline_tensor(istack_np, name="istack")

    ET = mybir.EngineType
    with tile.TileContext(nc) as tc:
        with (
            tc.tile_pool(name="const", bufs=1) as constp,
            tc.tile_pool(name="pa_pool", bufs=3) as pap,
            tc.tile_pool(name="fold", bufs=1) as foldp,
            tc.tile_pool(name="stat", bufs=1) as statp,
            tc.tile_pool(name="qx", bufs=8) as qxp,
            tc.tile_pool(name="psA", bufs=1, space="PSUM") as psa,
            tc.tile_pool(name="psB", bufs=1, space="PSUM") as psb,
        ):
            istack = constp.tile([128, R], f32)
            nc.sync.dma_start(out=istack[:, :], in_=istack_dram[:, :])

            # Stationary ring slots; off-diagonal zeros written once.
            slots = []
            for s in range(2):
                st = statp.tile([128, SLOT_F], f32, tag=f"slot{s}")
                q = SLOT_F // 4
                nc.vector.memset(st[:, 0:q], 0.0)
                nc.gpsimd.memset(st[:, q:2 * q], 0.0)
                nc.vector.memset(st[:, 2 * q:3 * q], 0.0)
                nc.gpsimd.memset(st[:, 3 * q:], 0.0)
                slots.append(st)

            loop_cm = (tc.For_i(0, bench_reps, 1,
                                hint_engines=(ET.PE, ET.SP, ET.Activation,
                                              ET.DVE, ET.Pool))
                       if bench else nullcontext())
            with loop_cm:
                # ---------- Phases A+B interleaved ---------------------------
                # Phase A (M-sum) matmuls are emitted in bursts of 8 between
                # chain step-layers: they keep the PE busy (and its pstate
                # high) while chain steps wait on their PSUM->SBUF copies.
                accs = [psa.tile([128, R], f32, space="PSUM", tag=f"acc{a}",
                                 name=f"acc{a}")
                        for a in range(PA_ACC)]
                total_mm = PA_TILES * PA_CHUNKS
                pav = pa[:, :, :].rearrange("r (t d v) s -> t d r (v s)",
                                            t=PA_TILES, d=4, v=PA_VB)
                a_tiles = {}
                state = {"n_mm": 0}

                def emit_a_dma(t):
                    if t >= PA_TILES or t in a_tiles:
                        return
                    ta = pap.tile([128, PA_F], f32, tag="pa_tile",
                                  name=f"pa_t{t}")
                    if new_pa:
                        nc.sync.dma_start(out=ta[:, :], in_=pav[t])
                    else:
                        for d in range(4):
                            v0 = t * PA_V + d * PA_VB
                            nc.sync.dma_start(
                                out=ta[32 * d:32 * (d + 1), :],
                                in_=pa[:, v0:v0 + PA_VB, :],
                            )
                    a_tiles[t] = ta

                def emit_a_units(count):
                    for _ in range(count):
                        n_mm = state["n_mm"]
                        if n_mm >= total_mm:
                            return
                        t, m = divmod(n_mm, PA_CHUNKS)
                        if m == 0:
                            emit_a_dma(t + 2)
                        f0 = m * 128
                        f1 = min(f0 + 128, PA_F)
                        a = n_mm % PA_ACC
                        nc.tensor.matmul(
                            out=accs[a][0:(f1 - f0), :],
                            lhsT=a_tiles[t][:, f0:f1],
                            rhs=istack[:, :],
                            start=(n_mm < PA_ACC),
                            stop=(n_mm >= total_mm - PA_ACC),
                            skip_group_check=True,
                        )
                        state["n_mm"] = n_mm + 1

                emit_a_dma(0)
                emit_a_dma(1)

                n_layers = (CHUNKS // 2) * SEG
                a_per_layer = -(-total_mm // n_layers)  # ceil -> 8

                for cp in range(CHUNKS // 2):
                    chunk_ids = (2 * cp, 2 * cp + 1)
                    for c in chunk_ids:
                        st = slots[c % 2]
                        stv = st[:, :].rearrange("p (t j s) -> p t j s",
                                                 t=TOK_PER_CJ, j=4, s=R)
                        for j in range(4):
                            nc.sync.dma_start(
                                out=stv[32 * j:32 * (j + 1), :, j, :],
                                in_=chain[c, j, :, :],
                            )
                    qprev = {}
                    for i in range(SEG):
                        psq = {}
                        for c in chunk_ids:
                            st = slots[c % 2]
                            for g4 in range(2):
                                ch = (c, g4)
                                ps = psb.tile([128, 128], f32, space="PSUM",
                                              tag=f"psq{c % 2}{g4}",
                                              name=f"psq_{c}_{g4}_{i}")
                                psq[ch] = ps
                                for k in range(4):
                                    qg = g4 * 4 + k
                                    tok = qg * SEG + i
                                    rhs = (istack[:, :] if i == 0
                                           else qprev[ch][:, 32 * k:32 * (k + 1)])
                                    nc.tensor.matmul(
                                        out=ps[:, 32 * k:32 * (k + 1)],
                                        lhsT=st[:, tok * 128:(tok + 1) * 128],
                                        rhs=rhs,
                                        start=True,
                                        stop=True,
                                    )
                        emit_a_units(a_per_layer)
                        for c in chunk_ids:
                            for g4 in range(2):
                                ch = (c, g4)
                                qnew = qxp.tile([128, 128], f32,
                                                tag=f"qq{c % 2}{g4}",
                                                name=f"qq_{c}_{g4}_{i}")
                                if g4 == 0:
                                    nc.vector.tensor_copy(out=qnew[:, :],
                                                          in_=psq[ch][:, :])
                                else:
                                    nc.scalar.copy(out=qnew[:, :],
                                                   in_=psq[ch][:, :])
                                qprev[ch] = qnew
                    for c in chunk_ids:
                        for g4 in range(2):
                            nc.sync.dma_start(out=q_out[c * 2 + g4, :, :],
                                              in_=qprev[(c, g4)][:, :])

                emit_a_units(total_mm)  # drain any remainder

                # Phase A tail: accs -> SBUF, pairwise add, fold via istack.
                aS = [foldp.tile([128, R], f32, tag=f"aS{a}", name=f"aS{a}")
                      for a in range(PA_ACC)]
                for a in range(PA_ACC):
                    nc.vector.tensor_copy(out=aS[a][:, :], in_=accs[a][:, :])
                nc.vector.tensor_add(out=aS[0][:, :], in0=aS[0][:, :],
                                     in1=aS[1][:, :])
                nc.vector.tensor_add(out=aS[2][:, :], in0=aS[2][:, :],
                                     in1=aS[3][:, :])
                nc.vector.tensor_add(out=aS[0][:, :], in0=aS[0][:, :],
                                     in1=aS[2][:, :])
                psumM = psa.tile([R, R], f32, space="PSUM", tag="acc0",
                                 name="psumM")
                nc.tensor.matmul(out=psumM[:, :], lhsT=istack[:, :],
                                 rhs=aS[0][:, :], start=True, stop=True)
                m_tile = foldp.tile([R, R], f32, tag="m_tile")
                nc.vector.tensor_copy(out=m_tile[:, :], in_=psumM[:, :])
                nc.sync.dma_start(out=m_out[:, :], in_=m_tile[:, :])

            if bench:
                dtile = constp.tile([1, 1], f32, tag="dt")
                nc.sync.dma_start(out=dtile[:, :], in_=tick[:, :])
                nc.sync.dma_start(out=done[:, :], in_=dtile[:, :])

    nc.compile()
    return nc


def _host_inputs(alpha, beta, core, label_ids):
    """Build per-core input maps. core: (R, V, R) f32; label_ids: (B, N) int."""
    core = np.ascontiguousarray(np.asarray(core, dtype=np.float32))
    lab = np.asarray(label_ids)

    in_maps = []
    for cidx in range(NCORES):
        pa = np.ascontiguousarray(core[:, cidx * VS:(cidx + 1) * VS, :])
        ch = np.empty((CHUNKS, 4, R, TOK_PER_CJ, R), dtype=np.float32)
        for c in range(CHUNKS):
            bb = c // 2
            half = c % 2
            b_global = BS * cidx + bb
            for j in range(4):
                segs = half * 32 + np.arange(QG_PER_CHUNK) * 4 + j
                pos = (segs[:, None] * SEG + np.arange(SEG)[None, :]).ravel()
                ys = lab[b_global, pos]
                ch[c, j] = core[:, ys, :]
        in_maps.append({
            "pa": pa,
            "chain": ch.reshape(CHUNKS, 4, R, TOK_PER_CJ * R),
        })
    return in_maps


def _host_finish(alpha, beta, m_parts, q_parts):
    """Combine per-core results into (loss, prob) with reference f32 semantics."""
    alpha = np.asarray(alpha, dtype=np.float32)
    beta = np.asarray(beta, dtype=np.float32)

    # M partial tiles are (s, r); sum cores then transpose to (r, s).
    M = np.zeros((R, R), dtype=np.float32)
    for mp in m_parts:
        M = M + np.asarray(mp).reshape(R, R).T.astype(np.float32)

    # Exact sequential norm chain (matches jax.lax.scan order).
    u = alpha.copy()
    for _ in range(N):
        u = (u @ M).astype(np.float32)
    norm = np.float32(u @ beta)

    # Segment products: q_parts[c] shape (8, 128, 128).
    prob_tilde = np.empty((B,), dtype=np.float32)
    with np.errstate(over="ignore", invalid="ignore"):
        for cidx in range(NCORES):
            qo = np.asarray(q_parts[cidx]).reshape(CHUNKS * 2, 128, 128)
            Q = {}
            for c in range(CHUNKS):
                bb = c // 2
                half = c % 2
                for g4 in range(2):
                    tileq = qo[c * 2 + g4]
                    for k in range(4):
                        qg = g4 * 4 + k
                        for j in range(4):
                            seg = half * 32 + qg * 4 + j
                            Q[(bb, seg)] = tileq[32 * j:32 * (j + 1),
                                                 32 * k:32 * (k + 1)]
            for bb in range(BS):
                v = alpha.copy()
                for seg in range(SEGS):
                    # Q_seg = P_seg^T ; v_row @ P_seg == Q_seg @ v_col
                    v = (Q[(bb, seg)] @ v).astype(np.float32)
                prob_tilde[BS * cidx + bb] = np.float32(v @ beta)

    with np.errstate(divide="ignore", invalid="ignore", over="ignore"):
        loss = np.float32(np.mean(-np.log(prob_tilde + EPS) + np.log(norm + EPS)))
        prob = (prob_tilde / norm).astype(np.float32)
    return loss, prob


_NC_CACHE = {}


def kernel(alpha, beta, core, label_ids):
    from concourse.bass_utils import run_bass_kernel_spmd

    if "nc" not in _NC_CACHE:
        _NC_CACHE["nc"] = build_nc()
    nc = _NC_CACHE["nc"]

    in_maps = _host_inputs(alpha, beta, core, label_ids)
    res = run_bass_kernel_spmd(nc, in_maps, core_ids=list(range(NCORES)))
    m_parts = [res.results[c]["m_out"] for c in range(NCORES)]
    q_parts = [res.results[c]["q_out"] for c in range(NCORES)]
    return _host_finish(alpha, beta, m_parts, q_parts)


# revision 19
# speedup vs baseline: 2.2740x; 2.2224x over previous
"""Trainium2 Bass kernel for nn_BasicTJDLayer (tensor-train joint distribution layer).

Reference computation (all f32):
    g_t = core[:, y_t, :]                 (B,N) token gathers of (R,R) slices
    v   = alpha; v = v @ g_t  (N steps)   -> prob_tilde[b] = v @ beta
    M   = core.sum(axis=1); u = alpha @ M^N -> norm = u @ beta
    loss = mean(-log(prob_tilde+eps) + log(norm+eps)); prob = prob_tilde/norm

Distribution over 8 NeuronCores (per the sharding hint: data-parallel over
batch; label_ids and the gathered core chain sharded over B; the core table
vocab-sharded for the normalization sum):
  - Phase A (M = sum over vocab): vocab-sharded, 4000 entries/core. Streamed
    as (128, 4000) tiles; PE matmuls against a stacked identity reduce the
    4 vocab sub-blocks, accumulating round-robin over 4 PSUM banks.
  - Phase B (token chains): batch-sharded, 2 batch rows/core. Each batch
    row's 1024-token chain is split into 64 segments of 16 tokens; segment
    products are computed on-device with 4 segments packed per 128-wide
    block-diagonal matmul (Q <- G^T Q). The host supplies the gathered
    chain (core[:, ys, :]); DMAs write only the diagonal 32x32 blocks of
    the per-step stationary tiles, whose off-diagonal zeros are memset
    once and never rewritten. Four step-chains run in lockstep (two
    chunks x two quad-quads), with PSUM->SBUF copies split DVE/ACT.
  - Host: assembles M from per-core partials, runs the exact sequential
    norm chain (matches jax.lax.scan order), and combines the 64 segment
    products per batch row (tiny O(B*S*R^2) glue).
"""

import numpy as np

R = 32
V = 32000
B = 16
N = 1024
EPS = np.float32(1e-10)

NCORES = 8
VS = V // NCORES            # 4000 vocab entries per core (phase A)
BS = B // NCORES            # 2 batch rows per core (phase B)
SEG = 16                    # tokens per segment
SEGS = N // SEG             # 64 segments per batch row
CHUNKS = 4                  # chain chunks per core = (batch row, half)
QG_PER_CHUNK = 8            # quad-groups (4 segments each) per chunk
TOK_PER_CJ = QG_PER_CHUNK * SEG           # 128 tokens per (chunk, j)
SLOT_F = TOK_PER_CJ * 128                 # 16384 f32 per partition per slot

# Phase A tiling
PA_TILES = 8                # tiles per core
PA_V = VS // PA_TILES       # 500 vocab entries per tile
PA_VB = PA_V // 4           # 125 per partition-block
PA_F = PA_VB * R            # 4000 f32 free per partition
PA_CHUNKS = (PA_F + 127) // 128   # 32 matmul chunks per tile (31 full + 1 of 32)
PA_ACC = 4                  # round-robin PSUM accumulators (new_pa)


def build_nc(bench_reps=None, new_pa=True, new_chain=True):
    """Build the SPMD program. bench_reps=None -> real kernel (external I/O);
    bench_reps=K -> timing variant: body wrapped in For_i(K) over Internal
    DRAM scratch, with a trivial external in/out pair."""
    from concourse import bass, bacc, mybir, tile
    from contextlib import nullcontext

    f32 = mybir.dt.float32
    bench = bench_reps is not None

    nc = bacc.Bacc(None, target_bir_lowering=False, debug=False)

    if bench:
        tick = nc.dram_tensor("tick", [1, 1], f32, kind="ExternalInput")
        pa = nc.dram_tensor("pa", [R, VS, R], f32)
        chain = nc.dram_tensor("chain", [CHUNKS, 4, R, TOK_PER_CJ * R], f32)
        m_out = nc.dram_tensor("m_out", [R, R], f32)
        q_out = nc.dram_tensor("q_out", [CHUNKS * 2, 128, 128], f32)
        done = nc.dram_tensor("done", [1, 1], f32, kind="ExternalOutput")
    else:
        pa = nc.dram_tensor("pa", [R, VS, R], f32, kind="ExternalInput")
        chain = nc.dram_tensor("chain", [CHUNKS, 4, R, TOK_PER_CJ * R], f32,
                               kind="ExternalInput")
        m_out = nc.dram_tensor("m_out", [R, R], f32, kind="ExternalOutput")
        q_out = nc.dram_tensor("q_out", [CHUNKS * 2, 128, 128], f32,
                               kind="ExternalOutput")

    # istack[32*d + r, m] = (r == m): phase-A reducer (moving) and the
    # stacked identity Q_0 for the first chain step of every segment.
    istack_np = np.tile(np.eye(R, dtype=np.float32), (4, 1))
    istack_dram = nc.inline_tensor(istack_np, name="istack")

    ET = mybir.EngineType
    with tile.TileContext(nc) as tc:
        with (
            tc.tile_pool(name="const", bufs=1) as constp,
            tc.tile_pool(name="pa_pool", bufs=3) as pap,
            tc.tile_pool(name="fold", bufs=1) as foldp,
            tc.tile_pool(name="stat", bufs=1) as statp,
            tc.tile_pool(name="qx", bufs=8) as qxp,
            tc.tile_pool(name="psA", bufs=1, space="PSUM") as psa,
            tc.tile_pool(name="psB", bufs=1, space="PSUM") as psb,
        ):
            istack = constp.tile([128, R], f32)
            nc.sync.dma_start(out=istack[:, :], in_=istack_dram[:, :])

            # Stationary ring slots; off-diagonal zeros written once.
            slots = []
            for s in range(2):
                st = statp.tile([128, SLOT_F], f32, tag=f"slot{s}")
                q = SLOT_F // 4
                nc.vector.memset(st[:, 0:q], 0.0)
                nc.gpsimd.memset(st[:, q:2 * q], 0.0)
                nc.vector.memset(st[:, 2 * q:3 * q], 0.0)
                nc.gpsimd.memset(st[:, 3 * q:], 0.0)
                slots.append(st)

            loop_cm = (tc.For_i(0, bench_reps, 1,
                                hint_engines=(ET.PE, ET.SP, ET.Activation,
                                              ET.DVE, ET.Pool))
                       if bench else nullcontext())
            with loop_cm:
                # ---------- Phases A+B interleaved ---------------------------
                # Phase A (M-sum) matmuls are emitted in bursts of 8 between
                # chain step-layers: they keep the PE busy (and its pstate
                # high) while chain steps wait on their PSUM->SBUF copies.
                accs = [psa.tile([128, R], f32, space="PSUM", tag=f"acc{a}",
                                 name=f"acc{a}")
                        for a in range(PA_ACC)]
                total_mm = PA_TILES * PA_CHUNKS
                pav = pa[:, :, :].rearrange("r (t d v) s -> t d r (v s)",
                                            t=PA_TILES, d=4, v=PA_VB)
                a_tiles = {}
                state = {"n_mm": 0}

                def emit_a_dma(t):
                    if t >= PA_TILES or t in a_tiles:
                        return
                    ta = pap.tile([128, PA_F], f32, tag="pa_tile",
                                  name=f"pa_t{t}")
                    if new_pa:
                        nc.sync.dma_start(out=ta[:, :], in_=pav[t])
                    else:
                        for d in range(4):
                            v0 = t * PA_V + d * PA_VB
                            nc.sync.dma_start(
                                out=ta[32 * d:32 * (d + 1), :],
                                in_=pa[:, v0:v0 + PA_VB, :],
                            )
                    a_tiles[t] = ta

                def emit_a_units(count):
                    for _ in range(count):
                        n_mm = state["n_mm"]
                        if n_mm >= total_mm:
                            return
                        t, m = divmod(n_mm, PA_CHUNKS)
                        if m == 0:
                            emit_a_dma(t + 2)
                        f0 = m * 128
                        f1 = min(f0 + 128, PA_F)
                        a = n_mm % PA_ACC
                        nc.tensor.matmul(
                            out=accs[a][0:(f1 - f0), :],
                            lhsT=a_tiles[t][:, f0:f1],
                            rhs=istack[:, :],
                            start=(n_mm < PA_ACC),
                            stop=(n_mm >= total_mm - PA_ACC),
                            skip_group_check=True,
                        )
                        state["n_mm"] = n_mm + 1

                emit_a_dma(0)
                emit_a_dma(1)

                n_layers = (CHUNKS // 2) * SEG
                a_per_layer = -(-total_mm // n_layers)  # ceil -> 8

                for cp in range(CHUNKS // 2):
                    chunk_ids = (2 * cp, 2 * cp + 1)
                    for c in chunk_ids:
                        st = slots[c % 2]
                        stv = st[:, :].rearrange("p (t j s) -> p t j s",
                                                 t=TOK_PER_CJ, j=4, s=R)
                        for j in range(4):
                            nc.sync.dma_start(
                                out=stv[32 * j:32 * (j + 1), :, j, :],
                                in_=chain[c, j, :, :],
                            )
                    qprev = {}
                    for i in range(SEG):
                        psq = {}
                        for c in chunk_ids:
                            st = slots[c % 2]
                            for g4 in range(2):
                                ch = (c, g4)
                                ps = psb.tile([128, 128], f32, space="PSUM",
                                              tag=f"psq{c % 2}{g4}",
                                              name=f"psq_{c}_{g4}_{i}")
                                psq[ch] = ps
                                for k in range(4):
                                    qg = g4 * 4 + k
                                    tok = qg * SEG + i
                                    rhs = (istack[:, :] if i == 0
                                           else qprev[ch][:, 32 * k:32 * (k + 1)])
                                    nc.tensor.matmul(
                                        out=ps[:, 32 * k:32 * (k + 1)],
                                        lhsT=st[:, tok * 128:(tok + 1) * 128],
                                        rhs=rhs,
                                        start=True,
                                        stop=True,
                                    )
                        emit_a_units(a_per_layer)
                        for c in chunk_ids:
                            for g4 in range(2):
                                ch = (c, g4)
                                qnew = qxp.tile([128, 128], f32,
                                                tag=f"qq{c % 2}{g4}",
                                                name=f"qq_{c}_{g4}_{i}")
                                if g4 == 0:
                                    nc.vector.tensor_copy(out=qnew[:, :],
                                                          in_=psq[ch][:, :])
                                else:
                                    nc.scalar.copy(out=qnew[:, :],
                                                   in_=psq[ch][:, :])
                                qprev[ch] = qnew
                    for c in chunk_ids:
                        for g4 in range(2):
                            nc.sync.dma_start(out=q_out[c * 2 + g4, :, :],
                                              in_=qprev[(c, g4)][:, :])

                emit_a_units(total_mm)  # drain any remainder

                # Phase A tail: accs -> SBUF, pairwise add, fold via istack.
                aS = [foldp.tile([128, R], f32, tag=f"aS{a}", name=f"aS{a}")
                      for a in range(PA_ACC)]
                for a in range(PA_ACC):
                    nc.vector.tensor_copy(out=aS[a][:, :], in_=accs[a][:, :])
                nc.vector.tensor_add(out=aS[0][:, :], in0=aS[0][:, :],
                                     in1=aS[1][:, :])
                nc.vector.tensor_add(out=aS[2][:, :], in0=aS[2][:, :],
                                     in1=aS[3][:, :])
                nc.vector.tensor_add(out=aS[0][:, :], in0=aS[0][:, :],
                                     in1=aS[2][:, :])
                psumM = psa.tile([R, R], f32, space="PSUM", tag="acc0",
                                 name="psumM")
                nc.tensor.matmul(out=psumM[:, :], lhsT=istack[:, :],
                                 rhs=aS[0][:, :], start=True, stop=True)
                m_tile = foldp.tile([R, R], f32, tag="m_tile")
                nc.vector.tensor_copy(out=m_tile[:, :], in_=psumM[:, :])
                nc.sync.dma_start(out=m_out[:, :], in_=m_tile[:, :])

            if bench:
                dtile = constp.tile([1, 1], f32, tag="dt")
                nc.sync.dma_start(out=dtile[:, :], in_=tick[:, :])
                nc.sync.dma_start(out=done[:, :], in_=dtile[:, :])

    nc.compile()
    return nc


def _host_inputs(alpha, beta, core, label_ids):
    """Build per-core input maps. core: (R, V, R) f32; label_ids: (B, N) int."""
    core = np.ascontiguousarray(np.asarray(core, dtype=np.float32))
    lab = np.asarray(label_ids)

    in_maps = []
    for cidx in range(NCORES):
        pa = np.ascontiguousarray(core[:, cidx * VS:(cidx + 1) * VS, :])
        ch = np.empty((CHUNKS, 4, R, TOK_PER_CJ, R), dtype=np.float32)
        for c in range(CHUNKS):
            bb = c // 2
            half = c % 2
            b_global = BS * cidx + bb
            for j in range(4):
                segs = half * 32 + np.arange(QG_PER_CHUNK) * 4 + j
                pos = (segs[:, None] * SEG + np.arange(SEG)[None, :]).ravel()
                ys = lab[b_global, pos]
                ch[c, j] = core[:, ys, :]
        in_maps.append({
            "pa": pa,
            "chain": ch.reshape(CHUNKS, 4, R, TOK_PER_CJ * R),
        })
    return in_maps


def _host_finish(alpha, beta, m_parts, q_parts):
    """Combine per-core results into (loss, prob) with reference f32 semantics."""
    alpha = np.asarray(alpha, dtype=np.float32)
    beta = np.asarray(beta, dtype=np.float32)

    # M partial tiles are (s, r); sum cores then transpose to (r, s).
    M = np.zeros((R, R), dtype=np.float32)
    for mp in m_parts:
        M = M + np.asarray(mp).reshape(R, R).T.astype(np.float32)

    # Exact sequential norm chain (matches jax.lax.scan order).
    u = alpha.copy()
    for _ in range(N):
        u = (u @ M).astype(np.float32)
    norm = np.float32(u @ beta)

    # Segment products: q_parts[c] shape (8, 128, 128).
    prob_tilde = np.empty((B,), dtype=np.float32)
    with np.errstate(over="ignore", invalid="ignore"):
        for cidx in range(NCORES):
            qo = np.asarray(q_parts[cidx]).reshape(CHUNKS * 2, 128, 128)
            Q = {}
            for c in range(CHUNKS):
                bb = c // 2
                half = c % 2
                for g4 in range(2):
                    tileq = qo[c * 2 + g4]
                    for k in range(4):
                        qg = g4 * 4 + k
                        for j in range(4):
                            seg = half * 32 + qg * 4 + j
                            Q[(bb, seg)] = tileq[32 * j:32 * (j + 1),
                                                 32 * k:32 * (k + 1)]
            for bb in range(BS):
                v = alpha.copy()
                for seg in range(SEGS):
                    # Q_seg = P_seg^T ; v_row @ P_seg == Q_seg @ v_col
                    v = (Q[(bb, seg)] @ v).astype(np.float32)
                prob_tilde[BS * cidx + bb] = np.float32(v @ beta)

    with np.errstate(divide="ignore", invalid="ignore", over="ignore"):
        loss = np.float32(np.mean(-np.log(prob_tilde + EPS) + np.log(norm + EPS)))
        prob = (prob_tilde / norm).astype(np.float32)
    return loss, prob


_NC_CACHE = {}


def kernel(alpha, beta, core, label_ids):
    from concourse.bass_utils import run_bass_kernel_spmd

    if "nc" not in _NC_CACHE:
        _NC_CACHE["nc"] = build_nc()
    nc = _NC_CACHE["nc"]

    in_maps = _host_inputs(alpha, beta, core, label_ids)
    res = run_bass_kernel_spmd(nc, in_maps, core_ids=list(range(NCORES)))
    m_parts = [res.results[c]["m_out"] for c in range(NCORES)]
    q_parts = [res.results[c]["q_out"] for c in range(NCORES)]
    return _host_finish(alpha, beta, m_parts, q_parts)
